# revision 16
# baseline (speedup 1.0000x reference)
"""Trainium2 8-core kernel for nn_AttnAgg (sparse attention aggregation).

Math (see reference):
  Q = main @ Wq.T + bq                     [2048, 512]
  K = other @ Wk.T + bk                    [2048, 512]
  attn = softmax(where(mask, -BIG, Q K.T / sqrt(512)), axis=-1)   [2048, 2048]
  out[b, m, k] = sum_o attn[m, o] * fix[b, o] * other[o, k]       [32, 2048, 512]

Sharding: rows of `main` (the m axis) are split 256-per-core across 8 cores —
attention and the big einsum shard perfectly with zero collectives; only the
K projection (~1 GFLOP) is replicated.

The dominant cost is the batched aggregation einsum (B*M*O*D = 137 GFLOP of
the ~144 GFLOP total).  It runs in fp8e4 with perf_mode=DoubleRow (2 fp8
MACs per PE cell per cycle; the DR matmuls issue at the 512-cycle streaming
floor).  Straight fp8 fails the 2e-2 tolerance (measured 2.9e-2), so the
batch coupling `fix` is mean/delta decomposed on the host:
fix[b,o] = mu[o] + delta[b,o].  The batch-independent mu-term
(p @ bf16(mu*other)) is ONE extra bf16 matmul pass (1/32 of the einsum
work); only the delta-term runs in fp8, and |p*delta| is ~half |p*fix|,
which halves the fp8 noise (measured 1.29e-2).  The mu-term is injected
into each batch's PSUM accumulation group by an identity-weight bf16
matmul that OPENS the group (start=True), so the psum->SBUF copy is a
plain per-partition recip scaling.  The softmax denominator comes from
the same bf16 p (matmul with ones), so normalization is consistent.
Projections run on bf16 inputs; the logits matmul stays float32r.

Per-batch steady state (period ~4.1us, PE ~3.9us busy):
  PE:  2 identity mean-add MMs + 16 DoubleRow MMs
  DVE: one merged 3D tensor_tensor wf[0:12] = pt * delta (stride-0
       broadcast of the delta column), + the mt1 output copy
  ACT: wf[12:16] (4 per-chunk activations, separate tile so the two wf
       write streams never serialize), + the mt0 output copy
Emission is software-pipelined LOOKAHEAD batches ahead so a copy stalled
on the PE never blocks the next batches' wf production (the engine queues
are strict FIFO).  Output staging is bf16 (host upcasts); outputs DMA out
in GB-batch groups.

Inputs are fed pre-transposed AND partition-packed: every DRAM tensor is
laid out [128, *] so that each SBUF partition's data is one long contiguous
DRAM run.  A tile row-block T of a logical [T*128, W] matrix lives at
packed[:, T*W:(T+1)*W]; for DoubleRow the pair dim indexes adjacent 128-row
blocks of the contraction (o) axis.
"""

import math
import os
import sys

import numpy as np

if "/opt/trn_rl_repo" not in sys.path:
    sys.path.insert(0, "/opt/trn_rl_repo")

import ml_dtypes

import concourse.bass as bass
import concourse.tile as tile
from concourse import bacc, mybir
from concourse.bass_utils import run_bass_kernel_spmd

F32 = mybir.dt.float32
F32R = mybir.dt.float32r
BF16 = mybir.dt.bfloat16
F8 = mybir.dt.float8e4
U8 = mybir.dt.uint8
AF = mybir.ActivationFunctionType
DR = mybir.MatmulPerfMode.DoubleRow

N_CORES = 8
M, O, D = 2048, 2048, 512       # main rows, other rows, qdim=kdim=mid
B = 32                          # batch
MC = M // N_CORES               # 256 main rows per core
P = 128
GB = 2                          # batches per output store DMA
N_WARM = 36                     # dummy matmuls to warm the PE clock gate
N_WF_DVE = 12                   # wf chunks (of 16) on DVE (one merged op)

_CACHE = {}
LAST_RESULTS = None             # test harness reads exec_time_ns from here


def _build():
    nc = bacc.Bacc("TRN2", target_bir_lowering=False, debug=False,
                   num_devices=N_CORES)

    NDT = D // P                # 4 tiles along the 512 dims
    NOT = O // P                # 16 tiles along o
    NMT = MC // P               # 2 tiles along m

    d_mainT = nc.dram_tensor("mainT", [P, NDT * MC], BF16,
                             kind="ExternalInput").ap()
    d_wqT = nc.dram_tensor("wqT", [P, NDT * D], BF16,
                           kind="ExternalInput").ap()
    d_bq = nc.dram_tensor("bq", [P, NDT], F32, kind="ExternalInput").ap()
    d_wkT = nc.dram_tensor("wkT", [P, NDT * D], BF16,
                           kind="ExternalInput").ap()
    d_bk = nc.dram_tensor("bk", [P, NDT], F32, kind="ExternalInput").ap()
    d_otherT = nc.dram_tensor("otherT", [P, NDT * O], BF16,
                              kind="ExternalInput").ap()   # fc-major
    d_other8 = nc.dram_tensor("other8", [P, NOT * D], F8,
                              kind="ExternalInput").ap()   # ot-major, fp8
    d_otherM = nc.dram_tensor("otherM", [P, NOT * D], BF16,
                              kind="ExternalInput").ap()   # mu*other, bf16
    d_deltaT = nc.dram_tensor("deltaT", [P, NOT * B], F32,
                              kind="ExternalInput").ap()   # fix - mu
    d_maskT = nc.dram_tensor("maskT", [P, NOT * MC], U8,
                             kind="ExternalInput").ap()
    d_ident = nc.dram_tensor("ident", [P, P], BF16,
                             kind="ExternalInput").ap()
    d_out = nc.dram_tensor("out", [MC, B, D], BF16,
                           kind="ExternalOutput").ap()

    with tile.TileContext(nc) as tc:
        with tc.tile_pool(name="persist", bufs=1) as pp, \
             tc.tile_pool(name="wpool", bufs=12) as wpool, \
             tc.tile_pool(name="outp", bufs=4) as outp:

            # ---- loads, in dependency order ---------------------------
            with tc.tile_pool(name="proj", bufs=1) as proj, \
                 tc.tile_pool(name="psqk", bufs=2, space="PSUM") as psqk:
                wkP = proj.tile([P, NDT * D], BF16, name="wkP", tag="wkP")
                nc.sync.dma_start(wkP[:, 0:P], d_wkT[:, 0:P])  # warmup gate
                nc.sync.dma_start(wkP[:, P:NDT * D], d_wkT[:, P:NDT * D])
                otP = proj.tile([P, NDT * O], BF16, name="otP", tag="otP")
                for ct in range(NDT):  # fc0 in ct-granular chunks: the first
                    nc.sync.dma_start(   # KT matmuls start ~3us earlier
                        otP[:, ct * D:(ct + 1) * D],
                        d_otherT[:, ct * D:(ct + 1) * D])
                wqP = proj.tile([P, NDT * D], BF16, name="wqP", tag="wqP")
                mtP = proj.tile([P, NDT * MC], BF16, name="mtP", tag="mtP")
                for ct in range(NDT):
                    nc.sync.dma_start(wqP[:, ct * D:(ct + 1) * D],
                                      d_wqT[:, ct * D:(ct + 1) * D])
                    nc.sync.dma_start(mtP[:, ct * MC:(ct + 1) * MC],
                                      d_mainT[:, ct * MC:(ct + 1) * MC])
                bqP = proj.tile([P, NDT], F32, name="bqP", tag="bqP")
                nc.sync.dma_start(bqP[:], d_bq[:])
                bkP = proj.tile([P, NDT], F32, name="bkP", tag="bkP")
                nc.sync.dma_start(bkP[:], d_bk[:])
                for fc in range(1, NDT):  # fc-major chunks pipeline with KT
                    nc.sync.dma_start(otP[:, fc * O:(fc + 1) * O],
                                      d_otherT[:, fc * O:(fc + 1) * O])
                maskP = pp.tile([P, NOT, MC], U8, name="maskP",
                                tag="maskP")
                nc.sync.dma_start(maskP[:], d_maskT[:])
                otherP = pp.tile([P, NOT, D], F8, name="otherP",
                                 tag="otherP")
                nc.sync.dma_start(otherP[:], d_other8[:])
                otherMP = pp.tile([P, NOT, D], BF16, name="otherMP",
                                  tag="otherMP")
                nc.sync.dma_start(otherMP[:], d_otherM[:])
                deltaP = pp.tile([P, NOT, B], F32, name="deltaP",
                                 tag="deltaP")
                nc.sync.dma_start(deltaP[:], d_deltaT[:])
                identP = pp.tile([P, P], BF16, name="identP", tag="identP")
                nc.sync.dma_start(identP[:], d_ident[:])

                qt_sb = [pp.tile([P, MC], F32, name=f"qt{i}", tag=f"qt{i}")
                         for i in range(NDT)]
                kt_sb = [pp.tile([P, O], F32, name=f"kt{i}", tag=f"kt{i}")
                         for i in range(NDT)]
                pt_all = pp.tile([P, NOT, MC], BF16, name="pt", tag="pt")
                ones_sb = pp.tile([P, 1], BF16, name="ones", tag="ones")
                nc.vector.memset(ones_sb[:], 1.0)
                recip_sb = [pp.tile([P, 1], F32, name=f"recip{i}",
                                    tag=f"recip{i}") for i in range(NMT)]
                mean_sb = [pp.tile([P, D], BF16, name=f"mean{i}",
                                   tag=f"mean{i}") for i in range(NMT)]

                # ---- PE warmup ----------------------------------------
                # Dummy matmuls gated only on the first DMA: they fill the
                # PE-idle window while the rest of the inputs stream in, so
                # the HAM clock-gate is at 8/8 when real work starts.
                warm_ps = psqk.tile([P, P], F32, name="warm_ps", tag="warm",
                                    bufs=1)
                for _ in range(N_WARM):
                    nc.tensor.matmul(warm_ps[:], wkP[:, 0:P], wkP[:, 0:P],
                                     start=True, stop=True)

                # ---- KT fc0 first (earliest DMAs), then QT, then rest
                def emit_qt():
                    for pt in range(NDT):
                        ps = psqk.tile([P, MC], F32, name="psq", tag="psq")
                        for ct in range(NDT):
                            nc.tensor.matmul(
                                ps[:],
                                wqP[:, ct * D + pt * P:ct * D + (pt + 1) * P],
                                mtP[:, ct * MC:(ct + 1) * MC],
                                start=(ct == 0), stop=(ct == NDT - 1))
                        nc.scalar.activation(qt_sb[pt][:].bitcast(F32R),
                                             ps[:], AF.Identity,
                                             bias=bqP[:, pt:pt + 1])

                for fc in [0, None, 1, 2, 3]:
                    if fc is None:
                        emit_qt()
                        continue
                    for pt in range(NDT):
                        ps = psqk.tile([P, D], F32, name="psk", tag="psk")
                        for ct in range(NDT):
                            nc.tensor.matmul(
                                ps[:],
                                wkP[:, ct * D + pt * P:ct * D + (pt + 1) * P],
                                otP[:, fc * O + ct * D:fc * O + (ct + 1) * D],
                                start=(ct == 0), stop=(ct == NDT - 1))
                        nc.scalar.activation(
                            kt_sb[pt][:, fc * D:(fc + 1) * D].bitcast(F32R),
                            ps[:], AF.Identity, bias=bkP[:, pt:pt + 1])

            # ---- attnT, exp, rowsum -----------------------------------
            # ps4 (attn: 2 + rowsum: 2 banks) and pso (out: 4 banks) coexist
            # so the first batch's matmuls need not wait for the softmax
            # tail to release PSUM — otherwise the PE goes idle long enough
            # mid-kernel for the HAM clock-gate to re-throttle it.
            with tc.tile_pool(name="ps4", bufs=2, space="PSUM") as ps4, \
                 tc.tile_pool(name="pso", bufs=4, space="PSUM") as psop:
                for op in range(NOT // 2):
                    ps = ps4.tile([P, 2, MC], F32, name="psa", tag="psa")
                    for j in range(2):
                        ot = 2 * op + j
                        for ct in range(NDT):
                            nc.tensor.matmul(
                                ps[:, j, :],
                                kt_sb[ct][:, ot * P:(ot + 1) * P]
                                .bitcast(F32R),
                                qt_sb[ct][:].bitcast(F32R),
                                start=(ct == 0), stop=(ct == NDT - 1))
                    # psa += mask * -1e9  (u8 -> f32 convert, scale, add in
                    # one DVE pass); exp underflows masked lanes to exactly 0
                    nc.vector.scalar_tensor_tensor(
                        ps[:], maskP[:, 2 * op:2 * op + 2, :], -1.0e9, ps[:],
                        op0=mybir.AluOpType.mult, op1=mybir.AluOpType.add)
                    nc.scalar.activation(pt_all[:, 2 * op:2 * op + 2, :],
                                         ps[:], AF.Exp)
                for mt in range(NMT):
                    ps = ps4.tile([P, 1], F32, name=f"psr{mt}", tag=f"psr{mt}",
                                  bufs=1)
                    for ot in range(NOT):
                        nc.tensor.matmul(
                            ps[:],
                            pt_all[:, ot, mt * P:(mt + 1) * P],
                            ones_sb[:],
                            start=(ot == 0), stop=(ot == NOT - 1))
                    nc.vector.reciprocal(recip_sb[mt][:], ps[:])

                # ---- mean term: meanS = recip * (pT.T @ (mu*other)) ---
                for mt in range(NMT):
                    ps = psop.tile([P, D], F32, name="pso", tag="pso")
                    for ot in range(NOT):
                        nc.tensor.matmul(
                            ps[:],
                            pt_all[:, ot, mt * P:(mt + 1) * P],
                            otherMP[:, ot, :],
                            start=(ot == 0), stop=(ot == NOT - 1))
                    nc.scalar.activation(mean_sb[mt][:], ps[:], AF.Copy)

                # ---- weighted aggregation (fp8 DoubleRow) -------------
                # Emission is software-pipelined one batch ahead: wf ops for
                # batch b enter the (in-order) DVE/ACT queues BEFORE the
                # psum->SBUF copies of batch b-1, so a copy stalled on the
                # PE never blocks the next batch's wf production.
                osb = {}
                wfs = {}

                def emit_wf(b):
                    wf = wfs[b] = wpool.tile([P, NOT, MC], F8, name="wf",
                                             tag="wf")
                    # DVE: one merged 3D op with stride-0 broadcast delta
                    nc.vector.tensor_tensor(
                        wf[:, 0:N_WF_DVE, :], pt_all[:, 0:N_WF_DVE, :],
                        deltaP[:, 0:N_WF_DVE, b:b + 1]
                        .to_broadcast([P, N_WF_DVE, MC]),
                        mybir.AluOpType.mult)
                    for ot in range(N_WF_DVE, NOT):
                        nc.scalar.activation(
                            wf[:, ot, :], pt_all[:, ot, :], AF.Copy,
                            scale=deltaP[:, ot, b:b + 1])

                def emit_agg(b):
                    wf = wfs.pop(b)
                    for mt in range(NMT):
                        if b % GB == 0:
                            osb[mt] = outp.tile([P, GB * D], BF16, name="osb",
                                                tag=f"osb{mt}")
                        ps = psop.tile([P, D], F32, name="pso", tag="pso")
                        # open the group with psum = mean (identity matmul),
                        # then accumulate the fp8 delta-term on top
                        nc.tensor.matmul(ps[:], identP[:], mean_sb[mt][:],
                                         start=True, stop=False,
                                         skip_group_check=True)
                        for op in range(NOT // 2):
                            nc.tensor.matmul(
                                ps[:],
                                wf[:, 2 * op:2 * op + 2,
                                   mt * P:(mt + 1) * P],
                                otherP[:, 2 * op:2 * op + 2, :],
                                start=False, stop=(op == NOT // 2 - 1),
                                perf_mode=DR, skip_group_check=True)
                        j = b % GB
                        nc.scalar.activation(osb[mt][:, j * D:(j + 1) * D],
                                             ps[:], AF.Copy,
                                             scale=recip_sb[mt][:])
                        if b >= B - GB:
                            # tail: store per-batch so the last DMA is small
                            nc.sync.dma_start(
                                d_out[mt * P:(mt + 1) * P, b:b + 1, :],
                                osb[mt][:, j * D:(j + 1) * D])
                        elif j == GB - 1:
                            nc.sync.dma_start(
                                d_out[mt * P:(mt + 1) * P,
                                      b - GB + 1:b + 1, :],
                                osb[mt][:])

                emit_wf(0)
                for b in range(B):
                    if b + 1 < B:
                        emit_wf(b + 1)
                    emit_agg(b)

    nc.compile()
    return nc


def _pack(a, ntiles, width):
    """[ntiles*128, width] -> [128, ntiles*width] partition-packed layout."""
    return np.ascontiguousarray(
        a.reshape(ntiles, P, width).transpose(1, 0, 2).reshape(P, -1))


def kernel(main_feat, other_feat, fix_feat, mask, Wq, bq, Wk, bk):
    global LAST_RESULTS
    main_feat = np.asarray(main_feat, dtype=np.float32)
    other_feat = np.asarray(other_feat, dtype=np.float32)
    fix_feat = np.asarray(fix_feat, dtype=np.float32)
    mask = np.asarray(mask)
    Wq = np.asarray(Wq, dtype=np.float32)
    bq = np.asarray(bq, dtype=np.float32)
    Wk = np.asarray(Wk, dtype=np.float32)
    bk = np.asarray(bk, dtype=np.float32)

    if "nc" not in _CACHE:
        _CACHE["nc"] = _build()
    nc = _CACHE["nc"]

    NDT, NOT = D // P, O // P
    inv = np.float32(1.0 / math.sqrt(D))
    wqT = _pack(Wq.T * inv, NDT, D).astype(ml_dtypes.bfloat16)
    bq_p = _pack((bq * inv).reshape(D, 1), NDT, 1)
    wkT = _pack(np.ascontiguousarray(Wk.T), NDT, D).astype(
        ml_dtypes.bfloat16)
    bk_p = _pack(bk.reshape(D, 1), NDT, 1)
    # otherT fc-major: [p, fc*O + ct*D + oo] = other.T[ct*128+p, fc*D+oo]
    otherT = np.ascontiguousarray(
        other_feat.T.reshape(NDT, P, NDT, D).transpose(1, 2, 0, 3)
        .reshape(P, NDT * O)).astype(ml_dtypes.bfloat16)
    other8 = _pack(other_feat, NOT, D).astype(ml_dtypes.float8_e4m3)
    mu = fix_feat.mean(axis=0)                        # [O]
    otherM = _pack(mu[:, None] * other_feat, NOT, D).astype(
        ml_dtypes.bfloat16)
    deltaT = _pack(np.ascontiguousarray((fix_feat - mu).T), NOT, B)
    ident = np.eye(P, dtype=np.float32).astype(ml_dtypes.bfloat16)
    mainT = main_feat.T                               # [D, M] view
    mask_u8 = mask.astype(np.uint8)                   # [M, O]

    in_maps = []
    for c in range(N_CORES):
        sl = slice(c * MC, (c + 1) * MC)
        in_maps.append({
            "mainT": _pack(np.ascontiguousarray(mainT[:, sl]),
                           NDT, MC).astype(ml_dtypes.bfloat16),
            "wqT": wqT, "bq": bq_p, "wkT": wkT, "bk": bk_p,
            "otherT": otherT, "other8": other8, "otherM": otherM,
            "deltaT": deltaT, "ident": ident,
            "maskT": _pack(np.ascontiguousarray(mask_u8[sl, :].T), NOT, MC),
        })

    try:
        res = run_bass_kernel_spmd(nc, in_maps, core_ids=list(range(N_CORES)))
    except Exception:
        # The BASS_TRACE=1 profiling path needs antenv.axon_hooks + artifact
        # upload, which not every image carries — rerun without tracing.
        if os.environ.get("BASS_NEVER_TRACE") == "1":
            raise
        os.environ["BASS_NEVER_TRACE"] = "1"
        res = run_bass_kernel_spmd(nc, in_maps, core_ids=list(range(N_CORES)))
    LAST_RESULTS = res
    # device layout is [MC, B, D] per core -> [B, MC, D], concat on m
    return np.concatenate(
        [res.results[c]["out"].transpose(1, 0, 2) for c in range(N_CORES)],
        axis=1).astype(np.float32)


# revision 17
# speedup vs baseline: 1.0846x; 1.0846x over previous
"""Trainium2 8-core kernel for nn_AttnAgg (sparse attention aggregation).

Math (see reference):
  Q = main @ Wq.T + bq                     [2048, 512]
  K = other @ Wk.T + bk                    [2048, 512]
  attn = softmax(where(mask, -BIG, Q K.T / sqrt(512)), axis=-1)   [2048, 2048]
  out[b, m, k] = sum_o attn[m, o] * fix[b, o] * other[o, k]       [32, 2048, 512]

Sharding: rows of `main` (the m axis) are split 256-per-core across 8 cores —
attention and the big einsum shard perfectly with zero collectives; only the
K projection (~1 GFLOP) is replicated.

The dominant cost is the batched aggregation einsum (B*M*O*D = 137 GFLOP of
the ~144 GFLOP total).  It runs in fp8e4 with perf_mode=DoubleRow (2 fp8
MACs per PE cell per cycle; the DR matmuls issue at the 512-cycle streaming
floor).  Straight fp8 fails the 2e-2 tolerance (measured 2.9e-2), so the
batch coupling `fix` is mean/delta decomposed on the host:
fix[b,o] = mu[o] + delta[b,o].  The batch-independent mu-term
(p @ bf16(mu*other)) is ONE extra bf16 matmul pass (1/32 of the einsum
work); only the delta-term runs in fp8, and |p*delta| is ~half |p*fix|,
which halves the fp8 noise (measured 1.29e-2).  The mu-term is injected
into each batch's PSUM accumulation group by an identity-weight bf16
matmul that OPENS the group (start=True), so the psum->SBUF copy is a
plain per-partition recip scaling.  The softmax denominator comes from
the same bf16 p (matmul with ones), so normalization is consistent.
Projections run on bf16 inputs; the logits matmul stays float32r.

Per-batch steady state (period ~4.1us, PE ~3.9us busy):
  PE:  2 identity mean-add MMs + 16 DoubleRow MMs
  DVE: one merged 3D tensor_tensor wf[0:12] = pt * delta (stride-0
       broadcast of the delta column), + the mt1 output copy
  ACT: wf[12:16] (4 per-chunk activations, separate tile so the two wf
       write streams never serialize), + the mt0 output copy
Emission is software-pipelined LOOKAHEAD batches ahead so a copy stalled
on the PE never blocks the next batches' wf production (the engine queues
are strict FIFO).  Output staging is bf16 (host upcasts); outputs DMA out
in GB-batch groups.

Inputs are fed pre-transposed AND partition-packed: every DRAM tensor is
laid out [128, *] so that each SBUF partition's data is one long contiguous
DRAM run.  A tile row-block T of a logical [T*128, W] matrix lives at
packed[:, T*W:(T+1)*W]; for DoubleRow the pair dim indexes adjacent 128-row
blocks of the contraction (o) axis.
"""

import math
import os
import sys

import numpy as np

if "/opt/trn_rl_repo" not in sys.path:
    sys.path.insert(0, "/opt/trn_rl_repo")

import ml_dtypes

import concourse.bass as bass
import concourse.tile as tile
from concourse import bacc, mybir
from concourse.bass_utils import run_bass_kernel_spmd

F32 = mybir.dt.float32
F32R = mybir.dt.float32r
BF16 = mybir.dt.bfloat16
F8 = mybir.dt.float8e4
U8 = mybir.dt.uint8
AF = mybir.ActivationFunctionType
DR = mybir.MatmulPerfMode.DoubleRow

N_CORES = 8
M, O, D = 2048, 2048, 512       # main rows, other rows, qdim=kdim=mid
B = 32                          # batch
MC = M // N_CORES               # 256 main rows per core
P = 128
GB = 2                          # batches per output store DMA
N_WARM = 36                     # dummy matmuls to warm the PE clock gate
N_WF_DVE = 12                   # wf chunks (of 16) on DVE (one merged op)

_CACHE = {}
LAST_RESULTS = None             # test harness reads exec_time_ns from here


def _build():
    nc = bacc.Bacc("TRN2", target_bir_lowering=False, debug=False,
                   num_devices=N_CORES)

    NDT = D // P                # 4 tiles along the 512 dims
    NOT = O // P                # 16 tiles along o
    NMT = MC // P               # 2 tiles along m

    d_mainT = nc.dram_tensor("mainT", [P, NDT * MC], BF16,
                             kind="ExternalInput").ap()
    d_wqT = nc.dram_tensor("wqT", [P, NDT * D], BF16,
                           kind="ExternalInput").ap()
    d_bq = nc.dram_tensor("bq", [P, NDT], F32, kind="ExternalInput").ap()
    d_wkT = nc.dram_tensor("wkT", [P, NDT * D], BF16,
                           kind="ExternalInput").ap()
    d_bk = nc.dram_tensor("bk", [P, NDT], F32, kind="ExternalInput").ap()
    d_otherT = nc.dram_tensor("otherT", [P, NDT * O], BF16,
                              kind="ExternalInput").ap()   # fc-major
    d_other8 = nc.dram_tensor("other8", [P, NOT * D], F8,
                              kind="ExternalInput").ap()   # ot-major, fp8
    d_otherM = nc.dram_tensor("otherM", [P, NOT * D], BF16,
                              kind="ExternalInput").ap()   # mu*other, bf16
    d_deltaT = nc.dram_tensor("deltaT", [P, NOT * B], F32,
                              kind="ExternalInput").ap()   # fix - mu
    d_maskT = nc.dram_tensor("maskT", [P, NOT * MC], U8,
                             kind="ExternalInput").ap()
    d_ident = nc.dram_tensor("ident", [P, P], BF16,
                             kind="ExternalInput").ap()
    d_out = nc.dram_tensor("out", [MC, B, D], BF16,
                           kind="ExternalOutput").ap()

    with tile.TileContext(nc) as tc:
        with tc.tile_pool(name="persist", bufs=1) as pp, \
             tc.tile_pool(name="wpool", bufs=14) as wpool, \
             tc.tile_pool(name="outp", bufs=6) as outp:

            # ---- loads, in dependency order ---------------------------
            with tc.tile_pool(name="proj", bufs=1) as proj, \
                 tc.tile_pool(name="psqk", bufs=3, space="PSUM") as psqk:
                wkP = proj.tile([P, NDT * D], BF16, name="wkP", tag="wkP")
                nc.sync.dma_start(wkP[:, 0:P], d_wkT[:, 0:P])  # warmup gate
                nc.sync.dma_start(wkP[:, P:NDT * D], d_wkT[:, P:NDT * D])
                otP = proj.tile([P, NDT * O], BF16, name="otP", tag="otP")
                for ct in range(NDT):  # fc0 in ct-granular chunks: the first
                    nc.sync.dma_start(   # KT matmuls start ~3us earlier
                        otP[:, ct * D:(ct + 1) * D],
                        d_otherT[:, ct * D:(ct + 1) * D])
                wqP = proj.tile([P, NDT * D], BF16, name="wqP", tag="wqP")
                mtP = proj.tile([P, NDT * MC], BF16, name="mtP", tag="mtP")
                for ct in range(NDT):
                    nc.sync.dma_start(wqP[:, ct * D:(ct + 1) * D],
                                      d_wqT[:, ct * D:(ct + 1) * D])
                    nc.sync.dma_start(mtP[:, ct * MC:(ct + 1) * MC],
                                      d_mainT[:, ct * MC:(ct + 1) * MC])
                bqP = proj.tile([P, NDT], F32, name="bqP", tag="bqP")
                nc.sync.dma_start(bqP[:], d_bq[:])
                bkP = proj.tile([P, NDT], F32, name="bkP", tag="bkP")
                nc.sync.dma_start(bkP[:], d_bk[:])
                for fc in range(1, NDT):  # fc-major chunks pipeline with KT
                    nc.sync.dma_start(otP[:, fc * O:(fc + 1) * O],
                                      d_otherT[:, fc * O:(fc + 1) * O])
                maskP = pp.tile([P, NOT, MC], U8, name="maskP",
                                tag="maskP")
                nc.sync.dma_start(maskP[:], d_maskT[:])
                otherP = pp.tile([P, NOT, D], F8, name="otherP",
                                 tag="otherP")
                nc.sync.dma_start(otherP[:], d_other8[:])
                otherMP = pp.tile([P, NOT, D], BF16, name="otherMP",
                                  tag="otherMP")
                nc.sync.dma_start(otherMP[:], d_otherM[:])
                deltaP = pp.tile([P, NOT, B], F32, name="deltaP",
                                 tag="deltaP")
                nc.sync.dma_start(deltaP[:], d_deltaT[:])
                identP = pp.tile([P, P], BF16, name="identP", tag="identP")
                nc.sync.dma_start(identP[:], d_ident[:])

                qt_sb = [pp.tile([P, MC], F32, name=f"qt{i}", tag=f"qt{i}")
                         for i in range(NDT)]
                kt_sb = [pp.tile([P, O], F32, name=f"kt{i}", tag=f"kt{i}")
                         for i in range(NDT)]
                pt_all = pp.tile([P, NOT, MC], BF16, name="pt", tag="pt")
                ones_sb = pp.tile([P, 1], BF16, name="ones", tag="ones")
                nc.vector.memset(ones_sb[:], 1.0)
                recip_sb = [pp.tile([P, 1], F32, name=f"recip{i}",
                                    tag=f"recip{i}") for i in range(NMT)]
                mean_sb = [pp.tile([P, D], BF16, name=f"mean{i}",
                                   tag=f"mean{i}") for i in range(NMT)]

                # ---- PE warmup ----------------------------------------
                # Dummy matmuls gated only on the first DMA: they fill the
                # PE-idle window while the rest of the inputs stream in, so
                # the HAM clock-gate is at 8/8 when real work starts.
                warm_ps = psqk.tile([P, P], F32, name="warm_ps", tag="warm",
                                    bufs=1)
                for _ in range(N_WARM):
                    nc.tensor.matmul(warm_ps[:], wkP[:, 0:P], wkP[:, 0:P],
                                     start=True, stop=True)

                # ---- KT fc0 first (earliest DMAs), then QT, then rest
                def emit_qt():
                    for pt in range(NDT):
                        ps = psqk.tile([P, MC], F32, name="psq", tag="psq")
                        for ct in range(NDT):
                            nc.tensor.matmul(
                                ps[:],
                                wqP[:, ct * D + pt * P:ct * D + (pt + 1) * P],
                                mtP[:, ct * MC:(ct + 1) * MC],
                                start=(ct == 0), stop=(ct == NDT - 1))
                        nc.scalar.activation(qt_sb[pt][:].bitcast(F32R),
                                             ps[:], AF.Identity,
                                             bias=bqP[:, pt:pt + 1])

                for fc in [0, None, 1, 2, 3]:
                    if fc is None:
                        emit_qt()
                        continue
                    for pt in range(NDT):
                        ps = psqk.tile([P, D], F32, name="psk", tag="psk")
                        for ct in range(NDT):
                            nc.tensor.matmul(
                                ps[:],
                                wkP[:, ct * D + pt * P:ct * D + (pt + 1) * P],
                                otP[:, fc * O + ct * D:fc * O + (ct + 1) * D],
                                start=(ct == 0), stop=(ct == NDT - 1))
                        nc.scalar.activation(
                            kt_sb[pt][:, fc * D:(fc + 1) * D].bitcast(F32R),
                            ps[:], AF.Identity, bias=bkP[:, pt:pt + 1])

            # ---- attnT, exp, rowsum -----------------------------------
            # ps4 (attn: 2 + rowsum: 2 banks) and pso (out: 4 banks) coexist
            # so the first batch's matmuls need not wait for the softmax
            # tail to release PSUM — otherwise the PE goes idle long enough
            # mid-kernel for the HAM clock-gate to re-throttle it.
            with tc.tile_pool(name="ps4", bufs=2, space="PSUM") as ps4, \
                 tc.tile_pool(name="pso", bufs=4, space="PSUM") as psop:
                for op in range(NOT // 2):
                    ps = ps4.tile([P, 2, MC], F32, name="psa", tag="psa")
                    for j in range(2):
                        ot = 2 * op + j
                        for ct in range(NDT):
                            nc.tensor.matmul(
                                ps[:, j, :],
                                kt_sb[ct][:, ot * P:(ot + 1) * P]
                                .bitcast(F32R),
                                qt_sb[ct][:].bitcast(F32R),
                                start=(ct == 0), stop=(ct == NDT - 1))
                    # psa += mask * -1e9  (u8 -> f32 convert, scale, add in
                    # one DVE pass); exp underflows masked lanes to exactly 0
                    nc.vector.scalar_tensor_tensor(
                        ps[:], maskP[:, 2 * op:2 * op + 2, :], -1.0e9, ps[:],
                        op0=mybir.AluOpType.mult, op1=mybir.AluOpType.add)
                    nc.scalar.activation(pt_all[:, 2 * op:2 * op + 2, :],
                                         ps[:], AF.Exp)
                for mt in range(NMT):
                    ps = ps4.tile([P, 1], F32, name=f"psr{mt}", tag=f"psr{mt}",
                                  bufs=1)
                    for ot in range(NOT):
                        nc.tensor.matmul(
                            ps[:],
                            pt_all[:, ot, mt * P:(mt + 1) * P],
                            ones_sb[:],
                            start=(ot == 0), stop=(ot == NOT - 1))
                    nc.vector.reciprocal(recip_sb[mt][:], ps[:])

                # ---- mean term: meanS = recip * (pT.T @ (mu*other)) ---
                for mt in range(NMT):
                    ps = psop.tile([P, D], F32, name="pso", tag="pso")
                    for ot in range(NOT):
                        nc.tensor.matmul(
                            ps[:],
                            pt_all[:, ot, mt * P:(mt + 1) * P],
                            otherMP[:, ot, :],
                            start=(ot == 0), stop=(ot == NOT - 1))
                    nc.scalar.activation(mean_sb[mt][:], ps[:], AF.Copy)

                # ---- weighted aggregation (fp8 DoubleRow) -------------
                # Emission is software-pipelined one batch ahead: wf ops for
                # batch b enter the (in-order) DVE/ACT queues BEFORE the
                # psum->SBUF copies of batch b-1, so a copy stalled on the
                # PE never blocks the next batch's wf production.
                osb = {}
                wfs = {}

                def emit_wf(b):
                    wf = wfs[b] = wpool.tile([P, NOT, MC], F8, name="wf",
                                             tag="wf")
                    # DVE: one merged 3D op with stride-0 broadcast delta
                    nc.vector.tensor_tensor(
                        wf[:, 0:N_WF_DVE, :], pt_all[:, 0:N_WF_DVE, :],
                        deltaP[:, 0:N_WF_DVE, b:b + 1]
                        .to_broadcast([P, N_WF_DVE, MC]),
                        mybir.AluOpType.mult)
                    for ot in range(N_WF_DVE, NOT):
                        nc.scalar.activation(
                            wf[:, ot, :], pt_all[:, ot, :], AF.Copy,
                            scale=deltaP[:, ot, b:b + 1])

                def emit_agg(b):
                    wf = wfs.pop(b)
                    for mt in range(NMT):
                        if b % GB == 0:
                            osb[mt] = outp.tile([P, GB * D], BF16, name="osb",
                                                tag=f"osb{mt}")
                        ps = psop.tile([P, D], F32, name="pso", tag="pso")
                        # open the group with psum = mean (identity matmul),
                        # then accumulate the fp8 delta-term on top
                        nc.tensor.matmul(ps[:], identP[:], mean_sb[mt][:],
                                         start=True, stop=False,
                                         skip_group_check=True)
                        for op in range(NOT // 2):
                            nc.tensor.matmul(
                                ps[:],
                                wf[:, 2 * op:2 * op + 2,
                                   mt * P:(mt + 1) * P],
                                otherP[:, 2 * op:2 * op + 2, :],
                                start=False, stop=(op == NOT // 2 - 1),
                                perf_mode=DR, skip_group_check=True)
                        j = b % GB
                        nc.scalar.activation(osb[mt][:, j * D:(j + 1) * D],
                                             ps[:], AF.Copy,
                                             scale=recip_sb[mt][:])
                        if b >= B - GB:
                            # tail: store per-batch so the last DMA is small
                            nc.sync.dma_start(
                                d_out[mt * P:(mt + 1) * P, b:b + 1, :],
                                osb[mt][:, j * D:(j + 1) * D])
                        elif j == GB - 1:
                            nc.sync.dma_start(
                                d_out[mt * P:(mt + 1) * P,
                                      b - GB + 1:b + 1, :],
                                osb[mt][:])

                emit_wf(0)
                for b in range(B):
                    if b + 1 < B:
                        emit_wf(b + 1)
                    emit_agg(b)

    nc.compile()
    return nc


def _pack(a, ntiles, width):
    """[ntiles*128, width] -> [128, ntiles*width] partition-packed layout."""
    return np.ascontiguousarray(
        a.reshape(ntiles, P, width).transpose(1, 0, 2).reshape(P, -1))


def kernel(main_feat, other_feat, fix_feat, mask, Wq, bq, Wk, bk):
    global LAST_RESULTS
    main_feat = np.asarray(main_feat, dtype=np.float32)
    other_feat = np.asarray(other_feat, dtype=np.float32)
    fix_feat = np.asarray(fix_feat, dtype=np.float32)
    mask = np.asarray(mask)
    Wq = np.asarray(Wq, dtype=np.float32)
    bq = np.asarray(bq, dtype=np.float32)
    Wk = np.asarray(Wk, dtype=np.float32)
    bk = np.asarray(bk, dtype=np.float32)

    if "nc" not in _CACHE:
        _CACHE["nc"] = _build()
    nc = _CACHE["nc"]

    NDT, NOT = D // P, O // P
    inv = np.float32(1.0 / math.sqrt(D))
    wqT = _pack(Wq.T * inv, NDT, D).astype(ml_dtypes.bfloat16)
    bq_p = _pack((bq * inv).reshape(D, 1), NDT, 1)
    wkT = _pack(np.ascontiguousarray(Wk.T), NDT, D).astype(
        ml_dtypes.bfloat16)
    bk_p = _pack(bk.reshape(D, 1), NDT, 1)
    # otherT fc-major: [p, fc*O + ct*D + oo] = other.T[ct*128+p, fc*D+oo]
    otherT = np.ascontiguousarray(
        other_feat.T.reshape(NDT, P, NDT, D).transpose(1, 2, 0, 3)
        .reshape(P, NDT * O)).astype(ml_dtypes.bfloat16)
    other8 = _pack(other_feat, NOT, D).astype(ml_dtypes.float8_e4m3)
    mu = fix_feat.mean(axis=0)                        # [O]
    otherM = _pack(mu[:, None] * other_feat, NOT, D).astype(
        ml_dtypes.bfloat16)
    deltaT = _pack(np.ascontiguousarray((fix_feat - mu).T), NOT, B)
    ident = np.eye(P, dtype=np.float32).astype(ml_dtypes.bfloat16)
    mainT = main_feat.T                               # [D, M] view
    mask_u8 = mask.astype(np.uint8)                   # [M, O]

    in_maps = []
    for c in range(N_CORES):
        sl = slice(c * MC, (c + 1) * MC)
        in_maps.append({
            "mainT": _pack(np.ascontiguousarray(mainT[:, sl]),
                           NDT, MC).astype(ml_dtypes.bfloat16),
            "wqT": wqT, "bq": bq_p, "wkT": wkT, "bk": bk_p,
            "otherT": otherT, "other8": other8, "otherM": otherM,
            "deltaT": deltaT, "ident": ident,
            "maskT": _pack(np.ascontiguousarray(mask_u8[sl, :].T), NOT, MC),
        })

    try:
        res = run_bass_kernel_spmd(nc, in_maps, core_ids=list(range(N_CORES)))
    except Exception:
        # The BASS_TRACE=1 profiling path needs antenv.axon_hooks + artifact
        # upload, which not every image carries — rerun without tracing.
        if os.environ.get("BASS_NEVER_TRACE") == "1":
            raise
        os.environ["BASS_NEVER_TRACE"] = "1"
        res = run_bass_kernel_spmd(nc, in_maps, core_ids=list(range(N_CORES)))
    LAST_RESULTS = res
    # device layout is [MC, B, D] per core -> [B, MC, D], concat on m
    return np.concatenate(
        [res.results[c]["out"].transpose(1, 0, 2) for c in range(N_CORES)],
        axis=1).astype(np.float32)
